# revision 74
# baseline (speedup 1.0000x reference)
"""Multi-head attention (B=8, N=1024, EMB=768, H=12, D=64) on 8 trn2 cores.

Strategy: data-parallel over batch (1 batch element per core, no collectives).

v4.5 (HAM-density rework):
  - x transposed + bf16 host-side; all matmul operands bf16; y computed
    transposed (y^T = w_out.T @ outT, host transposes back for free).
  - S psums are bf16 (single-shot matmuls, no accumulation) -> 1 PSUM bank
    each, bufs=3: S production double-buffers against the exp drain.
  - AV accumulates in query-halves [128,512] (1 bank each, bufs=3):
    normalization runs per half, so half-0 outT is final before half-1 AV
    runs; pair boundaries and the y tail pipeline instead of stalling.
    Keeping the PE dense avoids the HAM clock-throttle oscillation that
    cost ~38us at 1.2GHz in earlier versions.
  - AV stationary [V_h | ones x 64] (M=128): denominator lands in PSUM rows
    64:128 replicated; normalize = plain-op Newton reciprocal on DVE
    (XOR-seed + 1 NR step, ~0.4% max err) + fused sign-folding multiply.
  - startup: xT DMAs on the scalar queue; full-K warm-up matmuls bridge
    the HAM window while DMAs land.
"""

import numpy as np
import ml_dtypes
from contextlib import ExitStack

import concourse.bass as bass
import concourse.bacc as bacc
import concourse.tile as tile
from concourse import mybir
from concourse.bass_utils import run_bass_kernel_spmd

B, N, EMB = 8, 1024, 768
H, D = 12, 64
ATT = H * D          # 768
P = 128
NT = N // P          # 8 token chunks
EC = EMB // P        # 6 emb chunks
NP = H // 2          # 6 head pairs
FP = mybir.dt.float32
BF = mybir.dt.bfloat16
SCALE = 1.0 / float(np.sqrt(D))
HW = 512             # query half width

N_CORES = 8


def _emit_kernel(tc, xT_d, wqkv_d, wout_d, bout_d, y_d):
    nc = tc.nc
    with ExitStack() as ctx:
        const = ctx.enter_context(tc.tile_pool(name="const", bufs=1))
        ones_n = const.tile([1, HW], BF)
        nc.vector.memset(ones_n, 1.0)
        warm_sb = const.tile([P, HW], BF)
        nc.vector.memset(warm_sb, 0.03125)
        b_sb = const.tile([1, EMB], BF)
        nc.sync.dma_start(out=b_sb, in_=bout_d[:])

        outT_pool = ctx.enter_context(tc.tile_pool(name="outT", bufs=1,
                                                   side="right"))
        outT = [
            outT_pool.tile([P, N], BF, tag=f"outT{m}", name=f"outT{m}")
            for m in range(NP)
        ]
        wout_pool = ctx.enter_context(tc.tile_pool(name="wout", bufs=1,
                                                   side="right"))
        vaug_pool = ctx.enter_context(tc.tile_pool(name="vaugp", bufs=1,
                                                   side="right"))
        y_pool = ctx.enter_context(tc.tile_pool(name="y", bufs=1))

        with tc.tile_pool(name="weights", bufs=1) as wpool, \
             tc.tile_pool(name="att", bufs=1) as att, \
             tc.tile_pool(name="wvp", bufs=1) as wv_pool:

            strips = {}

            def emit_strip_dmas(p, fine=False):
                """Per-pair w_q/w_k column strips [128, 2, EC, 128]. One
                batched DMA per q/k normally; per-chunk DMAs for the
                startup-critical pair so the first matmuls start sooner."""
                st = wpool.tile([P, 2, EC, P], BF, tag="strip", bufs=3,
                                name=f"st{p}")
                for qk, col0 in ((0, p * P), (1, ATT + p * P)):
                    if fine:
                        # two k-triplet batches: 2 issues, and the first
                        # covers k=0..2 so the qkT accumulation starts early
                        for k0 in (0, 3):
                            nc.sync.dma_start(
                                out=st[:, qk, k0:k0 + 3, :],
                                in_=wqkv_d[k0 * P:(k0 + 3) * P,
                                           col0:col0 + P]
                                .rearrange("(k r) c -> r k c", r=P))
                    else:
                        nc.sync.dma_start(
                            out=st[:, qk, :, :],
                            in_=wqkv_d[:, col0:col0 + P]
                            .rearrange("(k r) c -> r k c", r=P))
                strips[p] = st

            ps = ctx.enter_context(tc.tile_pool(name="ps_main", bufs=1,
                                                space="PSUM"))

            # ---- PE warm-up: full-K matmuls engage the HAM un-throttle
            # (K=1 matmuls don't count as PE-busy) while startup DMAs run.
            ps_warm = ps.tile([P, N], FP, tag="ps", bufs=3, name="warm")
            for _ in range(12):
                nc.tensor.matmul(ps_warm[:, 0:HW], warm_sb[:, 0:P], warm_sb,
                                 start=True, stop=True)

            # ---- DMA: strips(0) on sync; xT on scalar queue (parallel);
            #      then wv, strips 1-2 on sync -------------------------
            emit_strip_dmas(0, fine=True)
            xt6 = wpool.tile([P, EC, N], BF, tag="xT6", name="xT6")
            for k in range(EC):
                nc.scalar.dma_start(out=xt6[:, k, :],
                                    in_=xT_d[k * P:(k + 1) * P, :])
            xT = [xt6[:, k, :] for k in range(EC)]
            wv6 = wv_pool.tile([P, EC, EMB], BF, tag="wv6", name="wv6")
            for k in range(EC):
                eng = nc.sync if k % 2 == 0 else nc.scalar
                eng.dma_start(out=wv6[:, k, :],
                              in_=wqkv_d[k * P:(k + 1) * P, 2 * ATT:])
            wv_sb = [wv6[:, k, :] for k in range(EC)]
            emit_strip_dmas(1)
            emit_strip_dmas(2)

            # ---- helper emitters ---------------------------------------
            class QKTEmitter:
                """Emits a pair's 24 qkT matmuls a few per call, so the PE
                gets steady filler work instead of a 24-MM burst (keeps the
                HAM activity monitor from re-throttling the clock)."""

                def __init__(self, p):
                    self.p = p
                    self.jobs = [(qk, nn, k)
                                 for qk in range(2)
                                 for nn in range(2)
                                 for k in range(EC)]
                    self.idx = 0
                    self.psq = None
                    self.tiles = [None, None]

                def done(self):
                    return self.idx >= len(self.jobs)

                def step(self, n=3):
                    while n > 0 and not self.done():
                        qk, nn, k = self.jobs[self.idx]
                        which = "qk"[qk]
                        if self.tiles[qk] is None:
                            self.tiles[qk] = wpool.tile(
                                [P, N], BF, tag=f"{which}Tp", bufs=3,
                                name=f"{which}T{self.p}")
                            self.psq = ps.tile([P, N], FP, tag="ps", bufs=3,
                                               name=f"ps{which}{self.p}")
                        nc.tensor.matmul(
                            self.psq[:, nn * HW:(nn + 1) * HW],
                            strips[self.p][:, qk, k, :],
                            xT[k][:, nn * HW:(nn + 1) * HW],
                            start=(k == 0),
                            stop=(k == EC - 1),
                        )
                        self.idx += 1
                        n -= 1
                        if k == EC - 1 and nn == 1:
                            nc.vector.tensor_copy(self.tiles[qk], self.psq)
                            self.psq = None
                    if self.done():
                        qkt[self.p] = self.tiles
                    return self.done()

            def emit_qkT(p):
                em = QKTEmitter(p)
                em.step(len(em.jobs))
                return qkt[p]

            def emit_v(t):
                # v psums ride the av tag (idle until the main loop) so the
                # ps rotation stays free for the S/exp pipeline.
                va = vaug_pool.tile([P, H, P], BF, tag=f"vaug{t}",
                                    name=f"vaug{t}")
                nc.gpsimd.memset(va[:, :, D:P], 1.0)
                for (n0, n1) in ((0, 512), (512, 768)):
                    psv = ps.tile([P, HW], FP, tag="av", bufs=2,
                                  name=f"psv{t}_{n0}")
                    w = n1 - n0
                    for k in range(EC):
                        nc.tensor.matmul(
                            psv[:, 0:w],
                            xT[k][:, t * P:(t + 1) * P],
                            wv_sb[k][:, n0:n1],
                            start=(k == 0),
                            stop=(k == EC - 1),
                        )
                    nc.vector.tensor_copy(
                        va[:, n0 // D:n1 // D, 0:D],
                        psv[:, 0:w].rearrange("p (h d) -> p h d", d=D),
                    )
                return va

            qkt = {}
            es_tiles = {}
            s_cursor = [0]

            def emit_S_unit(p, c, half):
                """S^T for key-chunk c, query-half `half`, BOTH heads in one
                PSUM tile (h0 -> cols 0:512 / row group 0, h1 -> cols
                512:1024 / row group 64). Sharing one tile means both
                matmuls' rotation dependency resolves together, so the
                row-group 2x packing engages every time; one exp covers
                both heads."""
                qT, kT = qkt[p]
                t = ps.tile([P, N], FP, tag="ps", bufs=3,
                            name=f"s{p}_{c}_{half}")
                for i, base in ((0, 0), (1, 64)):
                    nc.tensor.matmul(
                        t[:, i * HW:(i + 1) * HW],
                        kT[base:base + D, c * P:(c + 1) * P],
                        qT[base:base + D, half * HW:(half + 1) * HW],
                        start=True,
                        stop=True,
                    )
                es = att.tile([P, N], BF, tag="expS", bufs=30,
                              name=f"es{p}_{c}_{half}")
                nc.scalar.activation(
                    es, t, mybir.ActivationFunctionType.Exp, scale=SCALE)
                return es

            def pump_S(limit):
                """Emit S units (pair-major, half-major, then chunk) up to
                flat unit index `limit`."""
                while s_cursor[0] < min(limit, NP * NT * 2):
                    u = s_cursor[0]
                    p_, w = divmod(u, 2 * NT)
                    half_, c_ = divmod(w, NT)
                    if p_ not in qkt:
                        break
                    es_tiles[(p_, c_, half_)] = emit_S_unit(p_, c_, half_)
                    s_cursor[0] += 1

            def emit_normalize(p, i, half, av_t):
                # Plain-op Newton reciprocal: seed z0 = bitcast(~x)*c0 = -y0
                # (x*bitcast(~x) lands in [-4.5,-4] for any x>0), one NR step
                # z1 = (x*z0 + 2)*z0 = -y1, final multiply folds the sign:
                # outT = (raw * -1) * z1 = raw * y1.
                x = av_t[D:2 * D, :]               # [64,512] denom (repl)
                nx = att.tile([D, HW], FP, tag="nrm", bufs=4,
                              name=f"nx{p}_{i}_{half}")
                nc.vector.tensor_scalar(
                    out=nx.bitcast(mybir.dt.int32),
                    in0=x.bitcast(mybir.dt.int32),
                    scalar1=-1, scalar2=None,
                    op0=mybir.AluOpType.bitwise_xor)
                z0 = att.tile([D, HW], FP, tag="nrm", bufs=4,
                              name=f"z0{p}_{i}_{half}")
                nc.vector.tensor_scalar_mul(z0, nx, 0.23549792)
                pr = att.tile([D, HW], FP, tag="nrm", bufs=4,
                              name=f"pr{p}_{i}_{half}")
                nc.vector.tensor_mul(pr, x, z0)
                z1 = att.tile([D, HW], FP, tag="nrm", bufs=4,
                              name=f"z1{p}_{i}_{half}")
                nc.vector.scalar_tensor_tensor(
                    out=z1, in0=pr, scalar=-2.0, in1=z0,
                    op0=mybir.AluOpType.subtract,
                    op1=mybir.AluOpType.mult)
                nc.vector.scalar_tensor_tensor(
                    out=outT[p][i * D:(i + 1) * D,
                                half * HW:(half + 1) * HW],
                    in0=av_t[0:D, :], scalar=-1.0, in1=z1,
                    op0=mybir.AluOpType.mult,
                    op1=mybir.AluOpType.mult)

            # ---- prologue: qkT(0), then S(0) interleaved with v --------
            emit_qkT(0)
            vaug = []
            qkt1 = QKTEmitter(1)
            for c in range(NT):
                pump_S(2 * (c + 1))
                vaug.append(emit_v(c))
                qkt1.step(3)
                if c == 0:
                    emit_strip_dmas(3)
                if c == 4:
                    emit_strip_dmas(4)
            qkt1.step(len(qkt1.jobs))

            # w_out load rides the sync queue here (arrives mid-attention)
            wo6 = wout_pool.tile([P, EC, EMB], BF, tag="wout6", name="wout6")
            nc.sync.dma_start(
                out=wo6, in_=wout_d.rearrange("(k r) c -> r k c", r=P))
            wout_sb = [wo6[:, k, :] for k in range(EC)]

            # ---- y^T partial groups (k=0..4, both nn) on free ps tiles -
            ps_y = {}

            def emit_y_partial(e):
                t = ps.tile([P, N], FP, tag="ps", bufs=3, name=f"psy{e}")
                ps_y[e] = t
                for nn in range(2):
                    for k in range(NP - 1):
                        nc.tensor.matmul(
                            t[:, nn * HW:(nn + 1) * HW],
                            wout_sb[k][:, e * P:(e + 1) * P],
                            outT[k][:, nn * HW:(nn + 1) * HW],
                            start=(k == 0),
                            stop=False,
                        )

            y_nn0_done = set()

            def emit_y_nn0_finish(e):
                """k=5 + bias for the nn=0 query half — outT half 0 is final
                before the half-1 chains, so this runs during them."""
                t = ps_y[e]
                nc.tensor.matmul(
                    t[:, 0:HW],
                    wout_sb[NP - 1][:, e * P:(e + 1) * P],
                    outT[NP - 1][:, 0:HW],
                    start=False, stop=False,
                )
                nc.tensor.matmul(
                    t[:, 0:HW],
                    b_sb[0:1, e * P:(e + 1) * P],
                    ones_n,
                    start=False, stop=True,
                )
                y_nn0_done.add(e)

            def emit_y_finish(e):
                if e in ps_y:
                    t = ps_y[e]
                    k_range = [NP - 1]
                else:
                    t = ps.tile([P, N], FP, tag="ps", bufs=3,
                                name=f"psy{e}")
                    k_range = list(range(NP))
                for nn in range(2):
                    if nn == 0 and e in y_nn0_done:
                        continue
                    for k in k_range:
                        nc.tensor.matmul(
                            t[:, nn * HW:(nn + 1) * HW],
                            wout_sb[k][:, e * P:(e + 1) * P],
                            outT[k][:, nn * HW:(nn + 1) * HW],
                            start=(k == 0 and e not in ps_y),
                            stop=False,
                        )
                    nc.tensor.matmul(
                        t[:, nn * HW:(nn + 1) * HW],
                        b_sb[0:1, e * P:(e + 1) * P],  # K=1 bias matmul
                        ones_n,
                        start=False,
                        stop=True,
                    )
                y_sb = y_pool.tile([P, N], FP, tag="y", bufs=2,
                                   name=f"y{e}")
                nc.vector.tensor_copy(y_sb, t)
                nc.sync.dma_start(out=y_d[e * P:(e + 1) * P, :], in_=y_sb)

            # ---- main pair loop (AV per query-half) --------------------
            for p in range(NP):
                qkt_next = QKTEmitter(p + 2) if p + 2 < NP else None
                for half in range(2):
                    av_ts = [
                        ps.tile([P, HW], FP, tag="av", bufs=2,
                                name=f"av{p}_{i}_{half}")
                        for i in range(2)
                    ]
                    for c in range(NT):
                        # The PE queue is strict FIFO: put stall-prone S
                        # matmuls AFTER always-ready AV/qkT work, except at
                        # the pair boundary (c==0) where AV itself waits on
                        # the av-psum rotation and S is the filler.
                        if c == 0:
                            pump_S((p * 2 + half) * NT + c + 2 * NT + 6)
                        for i in range(2):
                            nc.tensor.matmul(
                                av_ts[i],
                                vaug[c][:, 2 * p + i, :],
                                es_tiles[(p, c, half)][:,
                                                       i * HW:(i + 1) * HW],
                                start=(c == 0),
                                stop=(c == NT - 1),
                            )
                        if qkt_next is not None and half == 0:
                            qkt_next.step(3)
                        if half == 0 and c == 4 and p == 0 and NP > 5:
                            emit_strip_dmas(5)
                        if p == NP - 1 and half == 1 and c in (1, 4, 7):
                            emit_y_partial({1: 0, 4: 1, 7: 2}[c])
                        if c > 0:
                            pump_S((p * 2 + half) * NT + c + 2 * NT + 6)
                    if qkt_next is not None and half == 0:
                        qkt_next.step(len(qkt_next.jobs))
                    emit_normalize(p, 0, half, av_ts[0])
                    emit_normalize(p, 1, half, av_ts[1])
                    # keep the HAM activity monitor fed across the
                    # normalize-chain stall (LDWEIGHTS needs no PSUM bank)
                    for _ in range(6):
                        nc.tensor.ldweights(weights=warm_sb[:, 0:P])

        # ---- output projection: finish partials, then remaining chunks -
        for e in range(EC):
            emit_y_finish(e)


_NC_CACHE = None


def _build_nc(reps=1):
    global _NC_CACHE
    if reps == 1 and _NC_CACHE is not None:
        return _NC_CACHE
    nc = bacc.Bacc("TRN2", target_bir_lowering=False, debug=False,
                   num_devices=N_CORES)
    xT_d = nc.declare_dram_parameter("xT", [EMB, N], BF, isOutput=False)
    wqkv_d = nc.declare_dram_parameter("w_qkv", [EMB, 3 * ATT], BF, isOutput=False)
    wout_d = nc.declare_dram_parameter("w_out", [ATT, EMB], BF, isOutput=False)
    bout_d = nc.declare_dram_parameter("b_out", [1, EMB], BF, isOutput=False)
    y_d = nc.declare_dram_parameter("y", [EMB, N], FP, isOutput=True)
    with tile.TileContext(nc) as tc:
        for _ in range(reps):
            _emit_kernel(tc, xT_d, wqkv_d, wout_d, bout_d, y_d)
    nc.compile()
    if reps == 1:
        _NC_CACHE = nc
    return nc


def run_sharded(x, w_qkv, w_out, b_out, **run_kwargs):
    """Shard over batch, run on 8 cores, gather. Returns (out, BassKernelResults)."""
    BFnp = ml_dtypes.bfloat16
    x = np.asarray(x, dtype=np.float32)
    w_qkv = np.asarray(w_qkv, dtype=np.float32).astype(BFnp)
    w_out = np.asarray(w_out, dtype=np.float32).astype(BFnp)
    b_out = np.asarray(b_out, dtype=np.float32).astype(BFnp).reshape(1, EMB)
    assert x.shape == (B, N, EMB)
    xT = [np.ascontiguousarray(x[i].T).astype(BFnp) for i in range(B)]
    nc = _build_nc()
    in_maps = [
        {"xT": xT[i], "w_qkv": w_qkv, "w_out": w_out, "b_out": b_out}
        for i in range(N_CORES)
    ]
    res = run_bass_kernel_spmd(nc, in_maps, core_ids=list(range(N_CORES)),
                               **run_kwargs)
    out = np.stack(
        [np.ascontiguousarray(res.results[i]["y"].T) for i in range(N_CORES)],
        axis=0)
    return out, res


def kernel(x, w_qkv, w_out, b_out):
    out, _ = run_sharded(x, w_qkv, w_out, b_out)
    return out


# revision 76
# speedup vs baseline: 1.1963x; 1.1963x over previous
"""Multi-head attention (B=8, N=1024, EMB=768, H=12, D=64) on 8 trn2 cores.

Strategy: data-parallel over batch (1 batch element per core, no collectives).

v4.5 (HAM-density rework):
  - x transposed + bf16 host-side; all matmul operands bf16; y computed
    transposed (y^T = w_out.T @ outT, host transposes back for free).
  - S psums are bf16 (single-shot matmuls, no accumulation) -> 1 PSUM bank
    each, bufs=3: S production double-buffers against the exp drain.
  - AV accumulates in query-halves [128,512] (1 bank each, bufs=3):
    normalization runs per half, so half-0 outT is final before half-1 AV
    runs; pair boundaries and the y tail pipeline instead of stalling.
    Keeping the PE dense avoids the HAM clock-throttle oscillation that
    cost ~38us at 1.2GHz in earlier versions.
  - AV stationary [V_h | ones x 64] (M=128): denominator lands in PSUM rows
    64:128 replicated; normalize = plain-op Newton reciprocal on DVE
    (XOR-seed + 1 NR step, ~0.4% max err) + fused sign-folding multiply.
  - startup: xT DMAs on the scalar queue; full-K warm-up matmuls bridge
    the HAM window while DMAs land.
"""

import numpy as np
import ml_dtypes
from contextlib import ExitStack

import concourse.bass as bass
import concourse.bacc as bacc
import concourse.tile as tile
from concourse import mybir
from concourse.bass_utils import run_bass_kernel_spmd

B, N, EMB = 8, 1024, 768
H, D = 12, 64
ATT = H * D          # 768
P = 128
NT = N // P          # 8 token chunks
EC = EMB // P        # 6 emb chunks
NP = H // 2          # 6 head pairs
FP = mybir.dt.float32
BF = mybir.dt.bfloat16
SCALE = 1.0 / float(np.sqrt(D))
HW = 512             # query half width

N_CORES = 8


def _emit_kernel(tc, xT_d, wqkv_d, wout_d, bout_d, y_d):
    nc = tc.nc
    with ExitStack() as ctx:
        const = ctx.enter_context(tc.tile_pool(name="const", bufs=1))
        ones_n = const.tile([1, HW], BF)
        nc.vector.memset(ones_n, 1.0)
        warm_sb = const.tile([P, HW], BF)
        nc.vector.memset(warm_sb, 0.03125)
        b_sb = const.tile([1, EMB], BF)
        nc.sync.dma_start(out=b_sb, in_=bout_d[:])

        outT_pool = ctx.enter_context(tc.tile_pool(name="outT", bufs=1,
                                                   side="right"))
        outT = [
            outT_pool.tile([P, N], BF, tag=f"outT{m}", name=f"outT{m}")
            for m in range(NP)
        ]
        wout_pool = ctx.enter_context(tc.tile_pool(name="wout", bufs=1,
                                                   side="right"))
        vaug_pool = ctx.enter_context(tc.tile_pool(name="vaugp", bufs=1,
                                                   side="right"))
        y_pool = ctx.enter_context(tc.tile_pool(name="y", bufs=1))

        with tc.tile_pool(name="weights", bufs=1) as wpool, \
             tc.tile_pool(name="att", bufs=1) as att, \
             tc.tile_pool(name="wvp", bufs=1) as wv_pool:

            strips = {}

            def emit_strip_dmas(p, fine=False):
                """Per-pair w_q/w_k column strips [128, 2, EC, 128]. One
                batched DMA per q/k normally; per-chunk DMAs for the
                startup-critical pair so the first matmuls start sooner."""
                st = wpool.tile([P, 2, EC, P], BF, tag="strip", bufs=3,
                                name=f"st{p}")
                for qk, col0 in ((0, p * P), (1, ATT + p * P)):
                    if fine:
                        for k in range(EC):
                            nc.sync.dma_start(
                                out=st[:, qk, k, :],
                                in_=wqkv_d[k * P:(k + 1) * P,
                                           col0:col0 + P])
                    else:
                        nc.sync.dma_start(
                            out=st[:, qk, :, :],
                            in_=wqkv_d[:, col0:col0 + P]
                            .rearrange("(k r) c -> r k c", r=P))
                strips[p] = st

            ps = ctx.enter_context(tc.tile_pool(name="ps_main", bufs=1,
                                                space="PSUM"))

            # ---- PE warm-up: full-K matmuls engage the HAM un-throttle
            # (K=1 matmuls don't count as PE-busy) while startup DMAs run.
            ps_warm = ps.tile([P, N], FP, tag="ps", bufs=3, name="warm")
            for _ in range(12):
                nc.tensor.matmul(ps_warm[:, 0:HW], warm_sb[:, 0:P], warm_sb,
                                 start=True, stop=True)

            # ---- DMA: strips(0) on sync; xT on scalar queue (parallel);
            #      then wv, strips 1-2 on sync -------------------------
            emit_strip_dmas(0, fine=True)
            xt6 = wpool.tile([P, EC, N], BF, tag="xT6", name="xT6")
            for k in range(EC):
                for (n0, n1) in ((0, 512), (512, N)):
                    nc.scalar.dma_start(
                        out=xt6[:, k, n0:n1],
                        in_=xT_d[k * P:(k + 1) * P, n0:n1])
            xT = [xt6[:, k, :] for k in range(EC)]
            wv6 = wv_pool.tile([P, EC, EMB], BF, tag="wv6", name="wv6")
            for k in range(EC):
                eng = nc.sync if k % 2 == 0 else nc.scalar
                eng.dma_start(out=wv6[:, k, :],
                              in_=wqkv_d[k * P:(k + 1) * P, 2 * ATT:])
            wv_sb = [wv6[:, k, :] for k in range(EC)]
            emit_strip_dmas(1)
            emit_strip_dmas(2)

            # ---- helper emitters ---------------------------------------
            class QKTEmitter:
                """Emits a pair's 24 qkT matmuls a few per call, so the PE
                gets steady filler work instead of a 24-MM burst (keeps the
                HAM activity monitor from re-throttling the clock)."""

                def __init__(self, p):
                    self.p = p
                    self.jobs = [(qk, nn, k)
                                 for qk in range(2)
                                 for nn in range(2)
                                 for k in range(EC)]
                    self.idx = 0
                    self.psq = None
                    self.tiles = [None, None]

                def done(self):
                    return self.idx >= len(self.jobs)

                def step(self, n=3):
                    while n > 0 and not self.done():
                        qk, nn, k = self.jobs[self.idx]
                        which = "qk"[qk]
                        if self.tiles[qk] is None:
                            self.tiles[qk] = wpool.tile(
                                [P, N], BF, tag=f"{which}Tp", bufs=3,
                                name=f"{which}T{self.p}")
                            self.psq = ps.tile([P, N], FP, tag="ps", bufs=3,
                                               name=f"ps{which}{self.p}")
                        nc.tensor.matmul(
                            self.psq[:, nn * HW:(nn + 1) * HW],
                            strips[self.p][:, qk, k, :],
                            xT[k][:, nn * HW:(nn + 1) * HW],
                            start=(k == 0),
                            stop=(k == EC - 1),
                        )
                        self.idx += 1
                        n -= 1
                        if k == EC - 1 and nn == 1:
                            nc.vector.tensor_copy(self.tiles[qk], self.psq)
                            self.psq = None
                    if self.done():
                        qkt[self.p] = self.tiles
                    return self.done()

            def emit_qkT(p):
                em = QKTEmitter(p)
                em.step(len(em.jobs))
                return qkt[p]

            def emit_v(t):
                # v psums ride the av tag (idle until the main loop) so the
                # ps rotation stays free for the S/exp pipeline.
                va = vaug_pool.tile([P, H, P], BF, tag=f"vaug{t}",
                                    name=f"vaug{t}")
                nc.gpsimd.memset(va[:, :, D:P], 1.0)
                for (n0, n1) in ((0, 512), (512, 768)):
                    psv = ps.tile([P, HW], FP, tag="av", bufs=2,
                                  name=f"psv{t}_{n0}")
                    w = n1 - n0
                    for k in range(EC):
                        nc.tensor.matmul(
                            psv[:, 0:w],
                            xT[k][:, t * P:(t + 1) * P],
                            wv_sb[k][:, n0:n1],
                            start=(k == 0),
                            stop=(k == EC - 1),
                        )
                    nc.vector.tensor_copy(
                        va[:, n0 // D:n1 // D, 0:D],
                        psv[:, 0:w].rearrange("p (h d) -> p h d", d=D),
                    )
                return va

            qkt = {}
            es_tiles = {}
            s_cursor = [0]

            def emit_S_unit(p, c, half):
                """S^T for key-chunk c, query-half `half`, BOTH heads in one
                PSUM tile (h0 -> cols 0:512 / row group 0, h1 -> cols
                512:1024 / row group 64). Sharing one tile means both
                matmuls' rotation dependency resolves together, so the
                row-group 2x packing engages every time; one exp covers
                both heads."""
                qT, kT = qkt[p]
                t = ps.tile([P, N], FP, tag="ps", bufs=3,
                            name=f"s{p}_{c}_{half}")
                for i, base in ((0, 0), (1, 64)):
                    nc.tensor.matmul(
                        t[:, i * HW:(i + 1) * HW],
                        kT[base:base + D, c * P:(c + 1) * P],
                        qT[base:base + D, half * HW:(half + 1) * HW],
                        start=True,
                        stop=True,
                    )
                es = att.tile([P, N], BF, tag="expS", bufs=30,
                              name=f"es{p}_{c}_{half}")
                nc.scalar.activation(
                    es, t, mybir.ActivationFunctionType.Exp, scale=SCALE)
                return es

            def pump_S(limit):
                """Emit S units (pair-major, half-major, then chunk) up to
                flat unit index `limit`."""
                while s_cursor[0] < min(limit, NP * NT * 2):
                    u = s_cursor[0]
                    p_, w = divmod(u, 2 * NT)
                    half_, c_ = divmod(w, NT)
                    if p_ not in qkt:
                        break
                    es_tiles[(p_, c_, half_)] = emit_S_unit(p_, c_, half_)
                    s_cursor[0] += 1

            def emit_normalize(p, i, half, av_t):
                # Plain-op Newton reciprocal: seed z0 = bitcast(~x)*c0 = -y0
                # (x*bitcast(~x) lands in [-4.5,-4] for any x>0), one NR step
                # z1 = (x*z0 + 2)*z0 = -y1, final multiply folds the sign:
                # outT = (raw * -1) * z1 = raw * y1.
                x = av_t[D:2 * D, :]               # [64,512] denom (repl)
                nx = att.tile([D, HW], FP, tag="nrm", bufs=4,
                              name=f"nx{p}_{i}_{half}")
                nc.vector.tensor_scalar(
                    out=nx.bitcast(mybir.dt.int32),
                    in0=x.bitcast(mybir.dt.int32),
                    scalar1=-1, scalar2=None,
                    op0=mybir.AluOpType.bitwise_xor)
                z0 = att.tile([D, HW], FP, tag="nrm", bufs=4,
                              name=f"z0{p}_{i}_{half}")
                nc.vector.tensor_scalar_mul(z0, nx, 0.23549792)
                pr = att.tile([D, HW], FP, tag="nrm", bufs=4,
                              name=f"pr{p}_{i}_{half}")
                nc.vector.tensor_mul(pr, x, z0)
                z1 = att.tile([D, HW], FP, tag="nrm", bufs=4,
                              name=f"z1{p}_{i}_{half}")
                nc.vector.scalar_tensor_tensor(
                    out=z1, in0=pr, scalar=-2.0, in1=z0,
                    op0=mybir.AluOpType.subtract,
                    op1=mybir.AluOpType.mult)
                nc.vector.scalar_tensor_tensor(
                    out=outT[p][i * D:(i + 1) * D,
                                half * HW:(half + 1) * HW],
                    in0=av_t[0:D, :], scalar=-1.0, in1=z1,
                    op0=mybir.AluOpType.mult,
                    op1=mybir.AluOpType.mult)

            # ---- prologue: qkT(0), then S(0) interleaved with v --------
            emit_qkT(0)
            vaug = []
            qkt1 = QKTEmitter(1)
            for c in range(NT):
                pump_S(2 * (c + 1))
                vaug.append(emit_v(c))
                qkt1.step(3)
                if c == 0:
                    emit_strip_dmas(3)
                if c == 4:
                    emit_strip_dmas(4)
            qkt1.step(len(qkt1.jobs))

            # w_out load rides the sync queue here (arrives mid-attention)
            wo6 = wout_pool.tile([P, EC, EMB], BF, tag="wout6", name="wout6")
            nc.sync.dma_start(
                out=wo6, in_=wout_d.rearrange("(k r) c -> r k c", r=P))
            wout_sb = [wo6[:, k, :] for k in range(EC)]

            # ---- y^T partial groups (k=0..4, both nn) on free ps tiles -
            ps_y = {}

            def emit_y_partial(e):
                t = ps.tile([P, N], FP, tag="ps", bufs=3, name=f"psy{e}")
                ps_y[e] = t
                for nn in range(2):
                    for k in range(NP - 1):
                        nc.tensor.matmul(
                            t[:, nn * HW:(nn + 1) * HW],
                            wout_sb[k][:, e * P:(e + 1) * P],
                            outT[k][:, nn * HW:(nn + 1) * HW],
                            start=(k == 0),
                            stop=False,
                        )

            y_nn0_done = set()

            def emit_y_nn0_finish(e):
                """k=5 + bias for the nn=0 query half — outT half 0 is final
                before the half-1 chains, so this runs during them."""
                t = ps_y[e]
                nc.tensor.matmul(
                    t[:, 0:HW],
                    wout_sb[NP - 1][:, e * P:(e + 1) * P],
                    outT[NP - 1][:, 0:HW],
                    start=False, stop=False,
                )
                nc.tensor.matmul(
                    t[:, 0:HW],
                    b_sb[0:1, e * P:(e + 1) * P],
                    ones_n,
                    start=False, stop=True,
                )
                y_nn0_done.add(e)

            def emit_y_finish(e):
                if e in ps_y:
                    t = ps_y[e]
                    k_range = [NP - 1]
                else:
                    t = ps.tile([P, N], FP, tag="ps", bufs=3,
                                name=f"psy{e}")
                    k_range = list(range(NP))
                for nn in range(2):
                    if nn == 0 and e in y_nn0_done:
                        continue
                    for k in k_range:
                        nc.tensor.matmul(
                            t[:, nn * HW:(nn + 1) * HW],
                            wout_sb[k][:, e * P:(e + 1) * P],
                            outT[k][:, nn * HW:(nn + 1) * HW],
                            start=(k == 0 and e not in ps_y),
                            stop=False,
                        )
                    nc.tensor.matmul(
                        t[:, nn * HW:(nn + 1) * HW],
                        b_sb[0:1, e * P:(e + 1) * P],  # K=1 bias matmul
                        ones_n,
                        start=False,
                        stop=True,
                    )
                y_sb = y_pool.tile([P, N], FP, tag="y", bufs=2,
                                   name=f"y{e}")
                nc.vector.tensor_copy(y_sb, t)
                nc.sync.dma_start(out=y_d[e * P:(e + 1) * P, :], in_=y_sb)

            # ---- main pair loop (AV per query-half) --------------------
            for p in range(NP):
                qkt_next = QKTEmitter(p + 2) if p + 2 < NP else None
                for half in range(2):
                    av_ts = [
                        ps.tile([P, HW], FP, tag="av", bufs=2,
                                name=f"av{p}_{i}_{half}")
                        for i in range(2)
                    ]
                    for c in range(NT):
                        # The PE queue is strict FIFO: put stall-prone S
                        # matmuls AFTER always-ready AV/qkT work, except at
                        # the pair boundary (c==0) where AV itself waits on
                        # the av-psum rotation and S is the filler.
                        if c == 0:
                            pump_S((p * 2 + half) * NT + c + 2 * NT + 6)
                        for i in range(2):
                            nc.tensor.matmul(
                                av_ts[i],
                                vaug[c][:, 2 * p + i, :],
                                es_tiles[(p, c, half)][:,
                                                       i * HW:(i + 1) * HW],
                                start=(c == 0),
                                stop=(c == NT - 1),
                            )
                        if qkt_next is not None and half == 0:
                            qkt_next.step(3)
                        if half == 0 and c == 4 and p == 0 and NP > 5:
                            emit_strip_dmas(5)
                        if p == NP - 1 and half == 1 and c in (1, 4, 7):
                            emit_y_partial({1: 0, 4: 1, 7: 2}[c])
                        if c > 0:
                            pump_S((p * 2 + half) * NT + c + 2 * NT + 6)
                    if qkt_next is not None and half == 0:
                        qkt_next.step(len(qkt_next.jobs))
                    emit_normalize(p, 0, half, av_ts[0])
                    emit_normalize(p, 1, half, av_ts[1])
                    # keep the HAM activity monitor fed across the
                    # normalize-chain stall (LDWEIGHTS needs no PSUM bank)
                    for _ in range(6):
                        nc.tensor.ldweights(weights=warm_sb[:, 0:P])

        # ---- output projection: finish partials, then remaining chunks -
        for e in range(EC):
            emit_y_finish(e)


_NC_CACHE = None


def _build_nc(reps=1):
    global _NC_CACHE
    if reps == 1 and _NC_CACHE is not None:
        return _NC_CACHE
    nc = bacc.Bacc("TRN2", target_bir_lowering=False, debug=False,
                   num_devices=N_CORES)
    xT_d = nc.declare_dram_parameter("xT", [EMB, N], BF, isOutput=False)
    wqkv_d = nc.declare_dram_parameter("w_qkv", [EMB, 3 * ATT], BF, isOutput=False)
    wout_d = nc.declare_dram_parameter("w_out", [ATT, EMB], BF, isOutput=False)
    bout_d = nc.declare_dram_parameter("b_out", [1, EMB], BF, isOutput=False)
    y_d = nc.declare_dram_parameter("y", [EMB, N], FP, isOutput=True)
    with tile.TileContext(nc) as tc:
        for _ in range(reps):
            _emit_kernel(tc, xT_d, wqkv_d, wout_d, bout_d, y_d)
    nc.compile()
    if reps == 1:
        _NC_CACHE = nc
    return nc


def run_sharded(x, w_qkv, w_out, b_out, **run_kwargs):
    """Shard over batch, run on 8 cores, gather. Returns (out, BassKernelResults)."""
    BFnp = ml_dtypes.bfloat16
    x = np.asarray(x, dtype=np.float32)
    w_qkv = np.asarray(w_qkv, dtype=np.float32).astype(BFnp)
    w_out = np.asarray(w_out, dtype=np.float32).astype(BFnp)
    b_out = np.asarray(b_out, dtype=np.float32).astype(BFnp).reshape(1, EMB)
    assert x.shape == (B, N, EMB)
    xT = [np.ascontiguousarray(x[i].T).astype(BFnp) for i in range(B)]
    nc = _build_nc()
    in_maps = [
        {"xT": xT[i], "w_qkv": w_qkv, "w_out": w_out, "b_out": b_out}
        for i in range(N_CORES)
    ]
    res = run_bass_kernel_spmd(nc, in_maps, core_ids=list(range(N_CORES)),
                               **run_kwargs)
    out = np.stack(
        [np.ascontiguousarray(res.results[i]["y"].T) for i in range(N_CORES)],
        axis=0)
    return out, res


def kernel(x, w_qkv, w_out, b_out):
    out, _ = run_sharded(x, w_qkv, w_out, b_out)
    return out
